# revision 1
# baseline (speedup 1.0000x reference)
"""Trainium kernel for nn_ActorTransformer_81862076662200.

Strategy: shard the atom axis A=512 across the 8 NeuronCores (64 atoms
per core).  The transformer attends across the graph axis B with the
atom axis as the MHA batch axis, so atom-sharding makes all six
encoder layers embarrassingly parallel — zero collectives.  The
sharded 6-layer encoder (>99% of FLOPs) runs on the 8 trn2 cores; the
tiny sequential tail (Set2Set pooling, memory LSTM, torsion gather,
final MLP, pad_sequence scatter — ~1.4 GFLOP total) runs on host in
numpy.
"""

import numpy as np

DIM = 256
HEADS = 8
LAYERS = 6
FF = 2048
ACTION = 6
STEPS = 6
B, A, T = 64, 512, 2016
N_CORES = 8
A_LOC = A // N_CORES

_compiled = {}


def _get_transformer():
    """Build (once) the pmapped atom-sharded 6-layer encoder."""
    if "fn" in _compiled:
        return _compiled["fn"]

    import jax
    import jax.numpy as jnp

    def _ln(x, g, b, eps=1e-5):
        m = x.mean(-1, keepdims=True)
        v = ((x - m) ** 2).mean(-1, keepdims=True)
        return (x - m) / jnp.sqrt(v + eps) * g + b

    def _mha(x, wqkv, bqkv, wo, bo):
        S, N, E = x.shape
        dh = E // HEADS
        q, k, v = jnp.split(x @ wqkv.T + bqkv, 3, axis=-1)
        q = q.reshape(S, N, HEADS, dh)
        k = k.reshape(S, N, HEADS, dh)
        v = v.reshape(S, N, HEADS, dh)
        s = jnp.einsum("snhd,tnhd->nhst", q, k) * (1.0 / np.sqrt(dh))
        a = jax.nn.softmax(s, axis=-1)
        o = jnp.einsum("nhst,tnhd->snhd", a, v).reshape(S, N, E)
        return o @ wo.T + bo

    def encoder(data_loc, lin0_w, lin0_b, qkv_w, qkv_b, out_w, out_b,
                ff1_w, ff1_b, ff2_w, ff2_b, ln1_g, ln1_b, ln2_g, ln2_b):
        # data_loc: (B, A_LOC, 3)
        x = jax.nn.relu(data_loc @ lin0_w.T + lin0_b)  # (B, A_LOC, DIM)
        for l in range(LAYERS):
            attn = _mha(x, qkv_w[l], qkv_b[l], out_w[l], out_b[l])
            x = _ln(x + attn, ln1_g[l], ln1_b[l])
            ff = jax.nn.relu(x @ ff1_w[l].T + ff1_b[l]) @ ff2_w[l].T + ff2_b[l]
            x = _ln(x + ff, ln2_g[l], ln2_b[l])
        return x

    devs = jax.devices()[:N_CORES]
    fn = jax.pmap(
        encoder,
        in_axes=(0,) + (None,) * 14,
        devices=devs,
    )
    _compiled["fn"] = fn
    return fn


def _np_sigmoid(x):
    return 1.0 / (1.0 + np.exp(-x))


def _lstm_step_np(x, h, c, wih, whh, bih, bhh):
    gates = x @ wih.T + bih + h @ whh.T + bhh
    i, f, g, o = np.split(gates, 4, axis=-1)
    c = _np_sigmoid(f) * c + _np_sigmoid(i) * np.tanh(g)
    h = _np_sigmoid(o) * np.tanh(c)
    return h, c


def kernel(data, nonring, nrbidx, torsion_list_sizes,
           lin0_w, lin0_b,
           enc_qkv_w, enc_qkv_b, enc_out_w, enc_out_b,
           enc_ff1_w, enc_ff1_b, enc_ff2_w, enc_ff2_b,
           enc_ln1_g, enc_ln1_b, enc_ln2_g, enc_ln2_b,
           s2s_wih, s2s_whh, s2s_bih, s2s_bhh,
           mem_wih, mem_whh, mem_bih, mem_bhh,
           lin1_w, lin1_b, lin2_w, lin2_b):
    data = np.asarray(data, np.float32)

    # ---- shard atoms contiguously: core i gets atoms [64*i, 64*i+64) ----
    data_sh = np.ascontiguousarray(
        data.reshape(B, N_CORES, A_LOC, 3).transpose(1, 0, 2, 3)
    )  # (8, B, A_LOC, 3)

    fn = _get_transformer()
    x_sh = fn(
        data_sh,
        np.asarray(lin0_w, np.float32), np.asarray(lin0_b, np.float32),
        np.asarray(enc_qkv_w, np.float32), np.asarray(enc_qkv_b, np.float32),
        np.asarray(enc_out_w, np.float32), np.asarray(enc_out_b, np.float32),
        np.asarray(enc_ff1_w, np.float32), np.asarray(enc_ff1_b, np.float32),
        np.asarray(enc_ff2_w, np.float32), np.asarray(enc_ff2_b, np.float32),
        np.asarray(enc_ln1_g, np.float32), np.asarray(enc_ln1_b, np.float32),
        np.asarray(enc_ln2_g, np.float32), np.asarray(enc_ln2_b, np.float32),
    )
    x_sh = np.asarray(x_sh)  # (8, B, A_LOC, DIM)

    # ---- unshard: (B, A, DIM) with atom index = core*A_LOC + local ----
    x3 = np.ascontiguousarray(x_sh.transpose(1, 0, 2, 3)).reshape(B, A, DIM)
    out_flat = x3.reshape(B * A, DIM)

    s2s_wih = np.asarray(s2s_wih, np.float32); s2s_whh = np.asarray(s2s_whh, np.float32)
    s2s_bih = np.asarray(s2s_bih, np.float32); s2s_bhh = np.asarray(s2s_bhh, np.float32)
    mem_wih = np.asarray(mem_wih, np.float32); mem_whh = np.asarray(mem_whh, np.float32)
    mem_bih = np.asarray(mem_bih, np.float32); mem_bhh = np.asarray(mem_bhh, np.float32)

    # ---- Set2Set pooling (6 steps) ----
    q_star = np.zeros((B, 2 * DIM), np.float32)
    h = np.zeros((B, DIM), np.float32)
    c = np.zeros((B, DIM), np.float32)
    for _ in range(STEPS):
        h, c = _lstm_step_np(q_star, h, c, s2s_wih, s2s_whh, s2s_bih, s2s_bhh)
        e = np.einsum("bad,bd->ba", x3, h)
        e = e - e.max(axis=1, keepdims=True)
        ex = np.exp(e)
        a = ex / ex.sum(axis=1, keepdims=True)
        r = np.einsum("ba,bad->bd", a, x3)
        q_star = np.concatenate([h, r], axis=-1).astype(np.float32)

    # ---- memory LSTM (single step, zero init) ----
    hx = np.zeros((B, DIM), np.float32)
    cx = np.zeros((B, DIM), np.float32)
    h1, c1 = _lstm_step_np(q_star, hx, cx, mem_wih, mem_whh, mem_bih, mem_bhh)
    h1 = h1.astype(np.float32); c1 = c1.astype(np.float32)

    # ---- torsion gather + final MLP ----
    nonring = np.asarray(nonring)
    nrbidx = np.asarray(nrbidx)
    lstm_sel = h1[nrbidx]  # (T, DIM)
    gathered = out_flat[nonring.reshape(-1)].reshape(4, T, DIM)
    stacked = np.concatenate([lstm_sel[None], gathered], 0)  # (5, T, DIM)
    feat = np.ascontiguousarray(np.transpose(stacked, (2, 1, 0))).reshape(-1, 5 * DIM)

    lin1_w = np.asarray(lin1_w, np.float32); lin1_b = np.asarray(lin1_b, np.float32)
    lin2_w = np.asarray(lin2_w, np.float32); lin2_b = np.asarray(lin2_b, np.float32)
    o = np.maximum(feat @ lin1_w.T + lin1_b, 0.0) @ lin2_w.T + lin2_b  # (T, ACTION)
    o = o.astype(np.float32)

    # ---- split by torsion_list_sizes + pad_sequence ----
    sizes = np.asarray(torsion_list_sizes)
    graph_ids = np.repeat(np.arange(B), sizes)
    offsets = np.concatenate([[0], np.cumsum(sizes)[:-1]])
    pos = np.arange(T) - offsets[graph_ids]
    max_s = int(sizes.max())
    logit = np.zeros((B, max_s, ACTION), o.dtype)
    logit[graph_ids, pos] = o
    return logit, h1[None], c1[None]


# revision 3
# speedup vs baseline: 70.0999x; 70.0999x over previous
"""Trainium kernel for nn_ActorTransformer_81862076662200.

Strategy: shard the atom axis A=512 across the 8 NeuronCores (64 atoms
per core).  The transformer attends across the graph axis B with the
atom axis as the MHA batch axis, so atom-sharding makes all six
encoder layers embarrassingly parallel — zero collectives.  The
sharded 6-layer encoder (>99% of FLOPs) runs on the 8 trn2 cores; the
tiny sequential tail (Set2Set pooling, memory LSTM, torsion gather,
final MLP, pad_sequence scatter — ~1.4 GFLOP total) runs on host in
numpy.  Weights are replicated to the cores once and cached.
"""

import numpy as np

DIM = 256
HEADS = 8
LAYERS = 6
FF = 2048
ACTION = 6
STEPS = 6
B, A, T = 64, 512, 2016
N_CORES = 8
A_LOC = A // N_CORES

_WNAMES = ["lin0_w", "lin0_b", "enc_qkv_w", "enc_qkv_b", "enc_out_w",
           "enc_out_b", "enc_ff1_w", "enc_ff1_b", "enc_ff2_w", "enc_ff2_b",
           "enc_ln1_g", "enc_ln1_b", "enc_ln2_g", "enc_ln2_b"]

_cache = {}


def _get_transformer():
    """Build (once) the pmapped atom-sharded 6-layer encoder."""
    if "fn" in _cache:
        return _cache["fn"]

    import jax
    import jax.numpy as jnp

    def _ln(x, g, b, eps=1e-5):
        m = x.mean(-1, keepdims=True)
        v = ((x - m) ** 2).mean(-1, keepdims=True)
        return (x - m) / jnp.sqrt(v + eps) * g + b

    def _mha(x, wqkv, bqkv, wo, bo):
        # x: (S, N, E); attends over S, batch axis is N (local atoms)
        S, N, E = x.shape
        dh = E // HEADS
        qkv = x @ wqkv.T + bqkv  # (S, N, 3E)
        q, k, v = jnp.split(qkv, 3, axis=-1)
        # (N*H, S, dh) batched layout for friendly batched matmuls
        q = q.reshape(S, N, HEADS, dh).transpose(1, 2, 0, 3).reshape(N * HEADS, S, dh)
        k = k.reshape(S, N, HEADS, dh).transpose(1, 2, 0, 3).reshape(N * HEADS, S, dh)
        v = v.reshape(S, N, HEADS, dh).transpose(1, 2, 0, 3).reshape(N * HEADS, S, dh)
        s = jnp.einsum("bsd,btd->bst", q, k) * (1.0 / np.sqrt(dh))
        a = jax.nn.softmax(s, axis=-1)
        o = jnp.einsum("bst,btd->bsd", a, v)  # (N*H, S, dh)
        o = o.reshape(N, HEADS, S, dh).transpose(2, 0, 1, 3).reshape(S, N, E)
        return o @ wo.T + bo

    def encoder(data_loc, lin0_w, lin0_b, qkv_w, qkv_b, out_w, out_b,
                ff1_w, ff1_b, ff2_w, ff2_b, ln1_g, ln1_b, ln2_g, ln2_b):
        # data_loc: (B, A_LOC, 3)
        x = jax.nn.relu(data_loc @ lin0_w.T + lin0_b)  # (B, A_LOC, DIM)
        for l in range(LAYERS):
            attn = _mha(x, qkv_w[l], qkv_b[l], out_w[l], out_b[l])
            x = _ln(x + attn, ln1_g[l], ln1_b[l])
            ff = jax.nn.relu(x @ ff1_w[l].T + ff1_b[l]) @ ff2_w[l].T + ff2_b[l]
            x = _ln(x + ff, ln2_g[l], ln2_b[l])
        return x

    devs = jax.devices()[:N_CORES]
    fn = jax.pmap(encoder, in_axes=(0,) * 15, devices=devs)
    _cache["fn"] = fn
    _cache["jax"] = jax
    return fn


def _device_weights(kw):
    """Replicate weights onto the 8 cores once; reuse across calls."""
    import jax
    key = tuple(id(kw[n]) for n in _WNAMES)
    if _cache.get("wkey") == key:
        return _cache["wrep"]
    devs = jax.devices()[:N_CORES]
    wrep = tuple(
        jax.device_put_replicated(np.asarray(kw[n], np.float32), devs)
        for n in _WNAMES
    )
    _cache["wkey"] = key
    _cache["wrep"] = wrep
    return wrep


def _np_sigmoid(x):
    return 1.0 / (1.0 + np.exp(-x))


def _lstm_step_np(x, h, c, wih, whh, bih, bhh):
    gates = x @ wih.T + bih + h @ whh.T + bhh
    i, f, g, o = np.split(gates, 4, axis=-1)
    c = _np_sigmoid(f) * c + _np_sigmoid(i) * np.tanh(g)
    h = _np_sigmoid(o) * np.tanh(c)
    return h, c


def kernel(data, nonring, nrbidx, torsion_list_sizes,
           lin0_w, lin0_b,
           enc_qkv_w, enc_qkv_b, enc_out_w, enc_out_b,
           enc_ff1_w, enc_ff1_b, enc_ff2_w, enc_ff2_b,
           enc_ln1_g, enc_ln1_b, enc_ln2_g, enc_ln2_b,
           s2s_wih, s2s_whh, s2s_bih, s2s_bhh,
           mem_wih, mem_whh, mem_bih, mem_bhh,
           lin1_w, lin1_b, lin2_w, lin2_b):
    data = np.asarray(data, np.float32)

    # ---- shard atoms contiguously: core i gets atoms [64*i, 64*i+64) ----
    data_sh = np.ascontiguousarray(
        data.reshape(B, N_CORES, A_LOC, 3).transpose(1, 0, 2, 3)
    )  # (8, B, A_LOC, 3)

    fn = _get_transformer()
    wrep = _device_weights(dict(
        lin0_w=lin0_w, lin0_b=lin0_b,
        enc_qkv_w=enc_qkv_w, enc_qkv_b=enc_qkv_b,
        enc_out_w=enc_out_w, enc_out_b=enc_out_b,
        enc_ff1_w=enc_ff1_w, enc_ff1_b=enc_ff1_b,
        enc_ff2_w=enc_ff2_w, enc_ff2_b=enc_ff2_b,
        enc_ln1_g=enc_ln1_g, enc_ln1_b=enc_ln1_b,
        enc_ln2_g=enc_ln2_g, enc_ln2_b=enc_ln2_b,
    ))
    x_sh = np.asarray(fn(data_sh, *wrep))  # (8, B, A_LOC, DIM)

    # ---- unshard: (B, A, DIM) with atom index = core*A_LOC + local ----
    x3 = np.ascontiguousarray(x_sh.transpose(1, 0, 2, 3)).reshape(B, A, DIM)
    out_flat = x3.reshape(B * A, DIM)

    s2s_wih = np.asarray(s2s_wih, np.float32); s2s_whh = np.asarray(s2s_whh, np.float32)
    s2s_bih = np.asarray(s2s_bih, np.float32); s2s_bhh = np.asarray(s2s_bhh, np.float32)
    mem_wih = np.asarray(mem_wih, np.float32); mem_whh = np.asarray(mem_whh, np.float32)
    mem_bih = np.asarray(mem_bih, np.float32); mem_bhh = np.asarray(mem_bhh, np.float32)

    # ---- Set2Set pooling (6 steps) ----
    q_star = np.zeros((B, 2 * DIM), np.float32)
    h = np.zeros((B, DIM), np.float32)
    c = np.zeros((B, DIM), np.float32)
    for _ in range(STEPS):
        h, c = _lstm_step_np(q_star, h, c, s2s_wih, s2s_whh, s2s_bih, s2s_bhh)
        e = np.einsum("bad,bd->ba", x3, h)
        e = e - e.max(axis=1, keepdims=True)
        ex = np.exp(e)
        a = ex / ex.sum(axis=1, keepdims=True)
        r = np.einsum("ba,bad->bd", a, x3)
        q_star = np.concatenate([h, r], axis=-1).astype(np.float32)

    # ---- memory LSTM (single step, zero init) ----
    hx = np.zeros((B, DIM), np.float32)
    cx = np.zeros((B, DIM), np.float32)
    h1, c1 = _lstm_step_np(q_star, hx, cx, mem_wih, mem_whh, mem_bih, mem_bhh)
    h1 = h1.astype(np.float32); c1 = c1.astype(np.float32)

    # ---- torsion gather + final MLP ----
    nonring = np.asarray(nonring)
    nrbidx = np.asarray(nrbidx)
    lstm_sel = h1[nrbidx]  # (T, DIM)
    gathered = out_flat[nonring.reshape(-1)].reshape(4, T, DIM)
    stacked = np.concatenate([lstm_sel[None], gathered], 0)  # (5, T, DIM)
    feat = np.ascontiguousarray(np.transpose(stacked, (2, 1, 0))).reshape(-1, 5 * DIM)

    lin1_w = np.asarray(lin1_w, np.float32); lin1_b = np.asarray(lin1_b, np.float32)
    lin2_w = np.asarray(lin2_w, np.float32); lin2_b = np.asarray(lin2_b, np.float32)
    o = np.maximum(feat @ lin1_w.T + lin1_b, 0.0) @ lin2_w.T + lin2_b  # (T, ACTION)
    o = o.astype(np.float32)

    # ---- split by torsion_list_sizes + pad_sequence ----
    sizes = np.asarray(torsion_list_sizes)
    graph_ids = np.repeat(np.arange(B), sizes)
    offsets = np.concatenate([[0], np.cumsum(sizes)[:-1]])
    pos = np.arange(T) - offsets[graph_ids]
    max_s = int(sizes.max())
    logit = np.zeros((B, max_s, ACTION), o.dtype)
    logit[graph_ids, pos] = o
    return logit, h1[None], c1[None]
